# revision 1
# baseline (speedup 1.0000x reference)
"""GCN layer on 8 Trainium2 NeuronCores.

Computes out = A @ (x @ W.T) where A is the sparse COO adjacency
(A[r, c] = sum of edge_vals over edges (r, c)), N=100000 nodes,
E=3200000 edges, D=256.

Strategy (1D destination partition, matmul-associativity reorder):
  out = (A @ x) @ W.T
- Destination nodes are sharded across the 8 cores (12500 rows each);
  x is replicated in every core's DRAM.
- Per core, edges are grouped on the host by dest-block (128 rows) and
  laid out in 128-edge tiles.  For each tile the kernel gathers the 128
  source rows x[col[e]] with one indirect DMA (one offset per
  partition), builds a scaled one-hot selection matrix
  S[e, d] = val[e] * (rowrel[e] == d) with a single DVE tensor_scalar,
  and accumulates S.T @ xg into a PSUM tile — a segmented sum over the
  128-row dest block.  G = (A @ x)-block is then transformed by W.T on
  the tensor engine (two transposes + two accumulating matmuls) and
  written out.
- SPMD: all cores run the identical program; per-dest-block tile counts
  are padded to the max across cores (offset 0 / val 0 filler).
"""

import numpy as np

P = 128
N = 100000
E = 3200000
D = 256
NCORES = 8
SH = N // NCORES          # 12500 dest rows per core
NDB = (SH + P - 1) // P   # 98 dest blocks (last one has 84 rows)


def _prep(edge_row, edge_col, edge_vals):
    """Group edges by (core, dest-block); build per-core DMA-ready offset /
    value / dest-row tables padded uniformly across cores."""
    core = edge_row // SH
    lrow = edge_row - core * SH
    db = lrow // P
    rowrel_all = (lrow % P).astype(np.float32)
    gkey = core.astype(np.int64) * NDB + db
    order = np.argsort(gkey, kind="stable")
    col_s = edge_col[order].astype(np.int32)
    val_s = edge_vals[order]
    row_s = rowrel_all[order]

    counts = np.bincount(gkey, minlength=NCORES * NDB).reshape(NCORES, NDB)
    starts = np.zeros(NCORES * NDB + 1, np.int64)
    np.cumsum(counts.ravel(), out=starts[1:])
    max_cnt = np.maximum(counts.max(axis=0), 1)          # [NDB]
    pad_to = ((max_cnt + P - 1) // P) * P                # [NDB]
    ttot = int(pad_to.sum()) // P                        # total edge tiles

    off_hosts, val_hosts, row_hosts = [], [], []
    for m in range(NCORES):
        off_h = np.zeros((P, ttot), np.int32)
        val_h = np.zeros((P, ttot), np.float32)
        row_h = np.zeros((P, ttot), np.float32)
        toff = 0
        for dbi in range(NDB):
            p = int(pad_to[dbi])
            t = p // P
            s0 = starts[m * NDB + dbi]
            cnt = int(counts[m, dbi])
            bo = np.zeros(p, np.int32)
            bo[:cnt] = col_s[s0 : s0 + cnt]
            bv = np.zeros(p, np.float32)
            bv[:cnt] = val_s[s0 : s0 + cnt]
            br = np.zeros(p, np.float32)
            br[:cnt] = row_s[s0 : s0 + cnt]
            off_h[:, toff : toff + t] = bo.reshape(t, P).T
            val_h[:, toff : toff + t] = bv.reshape(t, P).T
            row_h[:, toff : toff + t] = br.reshape(t, P).T
            toff += t
        off_hosts.append(off_h)
        val_hosts.append(val_h)
        row_hosts.append(row_h)

    return pad_to, ttot, off_hosts, val_hosts, row_hosts


def _build(pad_to, ttot, reps=1):
    """Build the SPMD bass program (identical on all cores).  reps>1 repeats
    the whole kernel body for in-NEFF benchmarking."""
    import concourse.bacc as bacc
    import concourse.bass as bass
    import concourse.mybir as mybir
    import concourse.tile as tile

    f32 = mybir.dt.float32
    i32 = mybir.dt.int32

    nc = bacc.Bacc("TRN2")
    x_d = nc.dram_tensor("x", [N, D], f32, kind="ExternalInput")
    wt_d = nc.dram_tensor("wt", [D, D], f32, kind="ExternalInput")
    iota_d = nc.dram_tensor("iota", [P, P], f32, kind="ExternalInput")
    ident_d = nc.dram_tensor("ident", [P, P], f32, kind="ExternalInput")
    off_d = nc.dram_tensor("off", [P, ttot], i32, kind="ExternalInput")
    val_d = nc.dram_tensor("val", [P, ttot], f32, kind="ExternalInput")
    row_d = nc.dram_tensor("row", [P, ttot], f32, kind="ExternalInput")
    out_d = nc.dram_tensor("out", [SH, D], f32, kind="ExternalOutput")

    with tile.TileContext(nc) as tc:
        with (
            tc.tile_pool(name="const", bufs=1) as constp,
            tc.tile_pool(name="meta", bufs=4) as metap,
            tc.tile_pool(name="gather", bufs=24) as gatherp,
            tc.tile_pool(name="s", bufs=8) as sp,
            tc.tile_pool(name="gsb", bufs=3) as gsbp,
            tc.tile_pool(name="osb", bufs=3) as osbp,
            tc.tile_pool(name="psg", bufs=2, space="PSUM") as psg,
            tc.tile_pool(name="pst", bufs=2, space="PSUM") as pst,
            tc.tile_pool(name="pso", bufs=2, space="PSUM") as pso,
        ):
            iota_t = constp.tile([P, P], f32)
            nc.sync.dma_start(out=iota_t[:], in_=iota_d[:])
            ident_t = constp.tile([P, P], f32)
            nc.sync.dma_start(out=ident_t[:], in_=ident_d[:])
            wt_t = []
            for k in range(2):
                w = constp.tile([P, D], f32, tag=f"wt{k}")
                nc.sync.dma_start(out=w[:], in_=wt_d[k * P : (k + 1) * P, :])
                wt_t.append(w)

            for _ in range(reps):
                toff = 0
                for dbi in range(NDB):
                    t_db = int(pad_to[dbi]) // P

                    off_t = metap.tile([P, t_db], i32, tag="off")
                    nc.sync.dma_start(
                        out=off_t[:], in_=off_d[:, toff : toff + t_db]
                    )
                    val_t = metap.tile([P, t_db], f32, tag="val")
                    nc.sync.dma_start(
                        out=val_t[:], in_=val_d[:, toff : toff + t_db]
                    )
                    row_t = metap.tile([P, t_db], f32, tag="row")
                    nc.sync.dma_start(
                        out=row_t[:], in_=row_d[:, toff : toff + t_db]
                    )

                    g_ps = psg.tile([P, D], f32)
                    for t in range(t_db):
                        xg = gatherp.tile([P, D], f32)
                        nc.gpsimd.indirect_dma_start(
                            out=xg[:],
                            out_offset=None,
                            in_=x_d[:],
                            in_offset=bass.IndirectOffsetOnAxis(
                                ap=off_t[:, t : t + 1], axis=0
                            ),
                        )
                        s_t = sp.tile([P, P], f32)
                        nc.vector.tensor_scalar(
                            out=s_t[:],
                            in0=iota_t[:],
                            scalar1=row_t[:, t : t + 1],
                            scalar2=val_t[:, t : t + 1],
                            op0=mybir.AluOpType.is_equal,
                            op1=mybir.AluOpType.mult,
                        )
                        nc.tensor.matmul(
                            g_ps[:],
                            lhsT=s_t[:],
                            rhs=xg[:],
                            start=(t == 0),
                            stop=(t == t_db - 1),
                        )

                    g_sb = gsbp.tile([P, D], f32)
                    nc.vector.tensor_copy(out=g_sb[:], in_=g_ps[:])
                    o_ps = pso.tile([P, D], f32)
                    for k in range(2):
                        t_ps = pst.tile([P, P], f32)
                        nc.tensor.transpose(
                            t_ps[:], g_sb[:, k * P : (k + 1) * P], ident_t[:]
                        )
                        gt_sb = gsbp.tile([P, P], f32, tag="gt")
                        nc.vector.tensor_copy(out=gt_sb[:], in_=t_ps[:])
                        nc.tensor.matmul(
                            o_ps[:],
                            lhsT=gt_sb[:],
                            rhs=wt_t[k][:],
                            start=(k == 0),
                            stop=(k == 1),
                        )
                    o_sb = osbp.tile([P, D], f32)
                    nc.vector.tensor_copy(out=o_sb[:], in_=o_ps[:])
                    rows = min(P, SH - dbi * P)
                    nc.sync.dma_start(
                        out=out_d[dbi * P : dbi * P + rows, :],
                        in_=o_sb[:rows, :],
                    )
                    toff += t_db

    nc.compile()
    return nc


def _make_in_maps(x, W, off_hosts, val_hosts, row_hosts):
    wt = np.ascontiguousarray(W.T)
    iota = np.tile(np.arange(P, dtype=np.float32), (P, 1))
    ident = np.eye(P, dtype=np.float32)
    return [
        {
            "x": x,
            "wt": wt,
            "iota": iota,
            "ident": ident,
            "off": off_hosts[m],
            "val": val_hosts[m],
            "row": row_hosts[m],
        }
        for m in range(NCORES)
    ]


def _run(nc, in_maps):
    from concourse.bass_utils import run_bass_kernel_spmd

    res = run_bass_kernel_spmd(nc, in_maps, list(range(NCORES)))
    return np.concatenate([res.results[m]["out"] for m in range(NCORES)], axis=0)


def kernel(x, W, edge_vals, edge_row, edge_col):
    x = np.asarray(x, np.float32)
    W = np.asarray(W, np.float32)
    edge_vals = np.asarray(edge_vals, np.float32)
    edge_row = np.asarray(edge_row, np.int32)
    edge_col = np.asarray(edge_col, np.int32)

    pad_to, ttot, off_hosts, val_hosts, row_hosts = _prep(
        edge_row, edge_col, edge_vals
    )
    nc = _build(pad_to, ttot, reps=1)
    in_maps = _make_in_maps(x, W, off_hosts, val_hosts, row_hosts)
    return _run(nc, in_maps)



# revision 15
# speedup vs baseline: 791.3529x; 791.3529x over previous
"""GCN layer on 8 Trainium2 NeuronCores.

Computes out = A @ (x @ W.T) where A is the sparse COO adjacency
(A[r, c] = sum of edge_vals over edges (r, c)), N=100000 nodes,
E=3200000 edges, D=256.

Strategy (1D destination partition, matmul-associativity reorder):
  out = (A @ x) @ W.T
- Destination nodes sharded across 8 cores (12500 rows each); x is
  replicated in every core's DRAM as bf16.
- Edges are grouped on the host by (dest-block of 128 rows, source
  chunk of 25000 rows) and padded to 128-edge tiles.  Source rows are
  fetched with batched `dma_gather` ops (one SWDGE op per source chunk
  per group of dest blocks, thousands of rows each) instead of one
  indirect DMA per 128-edge tile — this removes the ~1us/op SWDGE
  descriptor-generation serial bottleneck.  Rows are gathered as bf16
  (half the HBM traffic of fp32).
- For each 128-edge tile a scaled one-hot matrix
  S[e, d] = val[e] * (rowrel[e] == d) (bf16) is built with one DVE
  tensor_scalar.  Two accumulating matmuls per tile build the
  TRANSPOSED block partial gT_k[feat_k, dest] = xg_k.T @ S in PSUM
  (fp32), k = feature half.  This orientation removes the per-block
  transposes of the naive scheme: the output block is then
  o = sum_k gT_k.T @ W.T[k-half] via two more accumulating matmuls.
- SPMD: all cores run the identical program; per-(dest-block, chunk)
  tile counts are padded to the max across cores (index 0 / val 0
  filler, so no negative-index handling is needed).
"""

import numpy as np
import ml_dtypes

BF16 = ml_dtypes.bfloat16

P = 128
N = 100000
E = 3200000
D = 256
NCORES = 8
NCHUNK = 4                 # int16 gather indices: chunks of <= 32767 rows
G = 2                      # dest blocks per gather group


def _derived():
    sh = N // NCORES                    # dest rows per core
    ndb = (sh + P - 1) // P             # dest blocks per core
    ch = (N + NCHUNK - 1) // NCHUNK     # source rows per chunk
    assert ch <= 32767
    return sh, ndb, ch


class Plan:
    pass


def _prep(edge_row, edge_col, edge_vals):
    """Group edges by (core, dest-block, source-chunk); build per-core
    gather-index / val / rowrel tables padded uniformly across cores."""
    sh, ndb, ch = _derived()
    core = edge_row // sh
    lrow = edge_row - core * sh
    db = lrow // P
    rowrel = (lrow % P).astype(np.float32)
    chunk = edge_col // ch
    lcol = (edge_col - chunk * ch).astype(np.int16)

    gkey = ((core.astype(np.int64) * ndb + db) * NCHUNK + chunk)
    order = np.argsort(gkey, kind="stable")
    lcol_s = lcol[order]
    val_s = edge_vals[order]
    row_s = rowrel[order]

    counts = np.bincount(gkey, minlength=NCORES * ndb * NCHUNK).reshape(
        NCORES, ndb, NCHUNK
    )
    starts = np.zeros(NCORES * ndb * NCHUNK + 1, np.int64)
    np.cumsum(counts.ravel(), out=starts[1:])
    starts = starts.reshape(-1)

    # shared (across cores) tiles per (db, chunk)
    tile_cnt = (counts.max(axis=0) + P - 1) // P        # [ndb, NCHUNK]
    for dbi in range(ndb):
        if tile_cnt[dbi].sum() == 0:
            tile_cnt[dbi, 0] = 1                        # keep out rows defined

    # group layout: for each group of G dest blocks, ops are one per
    # chunk, tiles within an op ordered by db.
    ngrp = (ndb + G - 1) // G
    groups = []
    ttot = 0
    for g in range(ngrp):
        dbs = list(range(g * G, min((g + 1) * G, ndb)))
        goff = ttot
        ops = []        # (chunk, r0, r1, t0loc, topc)
        db_tiles = {dbi: [] for dbi in dbs}   # local tile cols per db
        tloc = 0
        for c in range(NCHUNK):
            t0loc = tloc
            for dbi in dbs:
                t = int(tile_cnt[dbi, c])
                db_tiles[dbi].extend(range(tloc, tloc + t))
                tloc += t
            topc = tloc - t0loc
            if topc > 0:
                ops.append((c, c * ch, min((c + 1) * ch, N), t0loc, topc))
        tg = tloc
        groups.append(
            dict(goff=goff, tg=tg, ops=ops,
                 dbs=[(dbi, db_tiles[dbi]) for dbi in dbs])
        )
        ttot += tg

    maxtg = max(gr["tg"] for gr in groups)

    idx_hosts, val_hosts, row_hosts = [], [], []
    for m in range(NCORES):
        idx_h = np.zeros((128, ttot * 8), np.int16)
        val_h = np.zeros((128, ttot), np.float32)
        row_h = np.zeros((128, ttot), np.float32)
        for g, gr in enumerate(groups):
            for (c, _r0, _r1, t0loc, topc) in gr["ops"]:
                n_op = topc * P
                idx_seq = np.zeros(n_op, np.int16)
                val_seq = np.zeros(n_op, np.float32)
                row_seq = np.zeros(n_op, np.float32)
                o = 0
                for dbi, _tl in gr["dbs"]:
                    t = int(tile_cnt[dbi, c])
                    if t == 0:
                        continue
                    s0 = starts[(m * (len(tile_cnt)) + dbi) * NCHUNK + c]
                    cnt = int(counts[m, dbi, c])
                    idx_seq[o : o + cnt] = lcol_s[s0 : s0 + cnt]
                    val_seq[o : o + cnt] = val_s[s0 : s0 + cnt]
                    row_seq[o : o + cnt] = row_s[s0 : s0 + cnt]
                    o += t * P
                assert o == n_op
                t0 = gr["goff"] + t0loc
                # gather-index wrapped layout: idx j -> [j%16, j//16],
                # replicated to all 8 gpsimd core slices
                w = idx_seq.reshape(-1, 16).T            # [16, n_op//16]
                idx_h[:, t0 * 8 : (t0 + topc) * 8] = np.tile(w, (8, 1))
                val_h[:, t0 : t0 + topc] = val_seq.reshape(topc, P).T
                row_h[:, t0 : t0 + topc] = row_seq.reshape(topc, P).T
        idx_hosts.append(idx_h)
        val_hosts.append(val_h)
        row_hosts.append(row_h)

    # split tiles: odd tile columns get host-precomputed S streamed from
    # DRAM; even columns are built on-chip (DVE, every 3rd on ACT)
    hoff = 0
    for gr in groups:
        htcols = []
        for dbi, tl in gr["dbs"]:
            htcols.extend(t for t in tl if (gr["goff"] + t) % 2 == 1)
        htcols = sorted(set(htcols))
        gr["htcols"] = {t: k for k, t in enumerate(htcols)}
        gr["hoff"] = hoff
        gr["nh"] = len(htcols)
        hoff += len(htcols)
    nh_tot = hoff

    s_hosts = []
    for m in range(NCORES):
        s_h = np.zeros((128, nh_tot * P), BF16)
        rows_all = row_hosts[m].astype(np.int32)
        vals_all = val_hosts[m]
        pidx = np.arange(128)
        for gr in groups:
            for t, k in gr["htcols"].items():
                tcol = gr["goff"] + t
                col0 = (gr["hoff"] + k) * P
                tile = np.zeros((128, P), np.float32)
                tile[pidx, rows_all[:, tcol]] = vals_all[:, tcol]
                s_h[:, col0 : col0 + P] = tile.astype(BF16)
        s_hosts.append(s_h)

    pl = Plan()
    pl.sh, pl.ndb, pl.ch = sh, ndb, ch
    pl.ttot, pl.maxtg, pl.groups = ttot, maxtg, groups
    pl.nh_tot = nh_tot
    pl.maxnh = max(gr["nh"] for gr in groups)
    pl.idx_hosts, pl.val_hosts, pl.row_hosts = idx_hosts, val_hosts, row_hosts
    pl.nval_hosts = [-v for v in val_hosts]
    pl.s_hosts = s_hosts
    return pl


def _build(pl, reps=1):
    """Build the SPMD bass program (identical on all cores)."""
    import concourse.bacc as bacc
    import concourse.mybir as mybir
    import concourse.tile as tile

    f32 = mybir.dt.float32
    bf16 = mybir.dt.bfloat16
    i16 = mybir.dt.int16

    sh, ndb, ttot = pl.sh, pl.ndb, pl.ttot

    nc = bacc.Bacc("TRN2", num_swdge_queues=4)
    x_d = nc.dram_tensor("x", [N, D], bf16, kind="ExternalInput")
    wt_d = nc.dram_tensor("wt", [D, D], bf16, kind="ExternalInput")
    iota_d = nc.dram_tensor("iota", [P, P], bf16, kind="ExternalInput")
    idx_d = nc.dram_tensor("idx", [128, ttot * 8], i16, kind="ExternalInput")
    val_d = nc.dram_tensor("val", [128, ttot], f32, kind="ExternalInput")
    row_d = nc.dram_tensor("row", [128, ttot], f32, kind="ExternalInput")
    nval_d = nc.dram_tensor("nval", [128, ttot], f32, kind="ExternalInput")
    sh_d = nc.dram_tensor("sh", [128, pl.nh_tot * P], bf16, kind="ExternalInput")
    out_d = nc.dram_tensor("out", [sh, D], f32, kind="ExternalOutput")

    with tile.TileContext(nc) as tc:
        with (
            tc.tile_pool(name="const", bufs=1) as constp,
            tc.tile_pool(name="meta", bufs=3) as metap,
            tc.tile_pool(name="xg", bufs=3) as xgp,
            tc.tile_pool(name="s", bufs=pl.maxtg) as sp,
            tc.tile_pool(name="shp", bufs=3) as shp,
            tc.tile_pool(name="z", bufs=16) as zp,
            tc.tile_pool(name="gtsb", bufs=8) as gtsbp,
            tc.tile_pool(name="osb", bufs=4) as osbp,
            tc.tile_pool(name="psg", bufs=3, space="PSUM") as psg,
            tc.tile_pool(name="pso", bufs=2, space="PSUM") as pso,
        ):
            iota_t = constp.tile([P, P], bf16)
            nc.sync.dma_start(out=iota_t[:], in_=iota_d[:])
            wt_t = []
            for k in range(2):
                w = constp.tile([P, D], bf16, tag=f"wt{k}")
                nc.sync.dma_start(out=w[:], in_=wt_d[k * P : (k + 1) * P, :])
                wt_t.append(w)

            for _ in range(reps):
                for gr in pl.groups:
                    goff, tg = gr["goff"], gr["tg"]
                    idx_t = metap.tile([128, tg * 8], i16, tag="idx")
                    nc.sync.dma_start(
                        out=idx_t[:], in_=idx_d[:, goff * 8 : (goff + tg) * 8]
                    )
                    val_t = metap.tile([128, tg], f32, tag="val")
                    nc.sync.dma_start(
                        out=val_t[:], in_=val_d[:, goff : goff + tg]
                    )
                    row_t = metap.tile([128, tg], f32, tag="row")
                    nc.sync.dma_start(
                        out=row_t[:], in_=row_d[:, goff : goff + tg]
                    )
                    nval_t = metap.tile([128, tg], f32, tag="nval")
                    nc.sync.dma_start(
                        out=nval_t[:], in_=nval_d[:, goff : goff + tg]
                    )

                    sh_t = shp.tile([128, max(gr["nh"], 1) * P], bf16, tag="sh")
                    if gr["nh"]:
                        nc.sync.dma_start(
                            out=sh_t[:],
                            in_=sh_d[:, gr["hoff"] * P : (gr["hoff"] + gr["nh"]) * P],
                        )
                    xg = xgp.tile([128, tg * D], bf16, tag="xg")
                    for (c, r0, r1, t0loc, topc) in gr["ops"]:
                        nc.gpsimd.dma_gather(
                            xg[:, t0loc * D : (t0loc + topc) * D].rearrange(
                                "p (t e) -> p t e", e=D
                            ),
                            x_d[r0:r1, :],
                            idx_t[:, t0loc * 8 : (t0loc + topc) * 8],
                            topc * P,
                            topc * P,
                            D,
                            single_packet=False,
                            queue_num=c,
                        )

                    # build all S tiles of the group first so the DVE/ACT
                    # streams never stall behind PE-dependent PSUM copies
                    s_tiles = {}
                    for k, (dbi, tcols) in enumerate(gr["dbs"]):
                        for j, tcol in enumerate(tcols):
                            if tcol in gr["htcols"]:
                                hk = gr["htcols"][tcol]
                                s_tiles[(dbi, j)] = sh_t[:, hk * P : (hk + 1) * P]
                                continue
                            s_t = sp.tile([P, P], bf16)
                            nc.vector.tensor_scalar(
                                out=s_t[:],
                                in0=iota_t[:],
                                scalar1=row_t[:, tcol : tcol + 1],
                                scalar2=val_t[:, tcol : tcol + 1],
                                op0=mybir.AluOpType.is_equal,
                                op1=mybir.AluOpType.mult,
                            )
                            s_tiles[(dbi, j)] = s_t

                    for dbi, tcols in gr["dbs"]:
                        gt0 = psg.tile([P, P], f32, tag="gt0")
                        gt1 = psg.tile([P, P], f32, tag="gt1")
                        last = len(tcols) - 1
                        for j, tcol in enumerate(tcols):
                            s_t = s_tiles[(dbi, j)]
                            nc.tensor.matmul(
                                gt0[:],
                                lhsT=xg[:, tcol * D : tcol * D + P],
                                rhs=s_t[:],
                                start=(j == 0),
                                stop=(j == last),
                            )
                            nc.tensor.matmul(
                                gt1[:],
                                lhsT=xg[:, tcol * D + P : tcol * D + 2 * P],
                                rhs=s_t[:],
                                start=(j == 0),
                                stop=(j == last),
                            )
                        g_sb = []
                        for k, gt in enumerate((gt0, gt1)):
                            gsb = gtsbp.tile([P, P], bf16, tag=f"g{k}")
                            nc.scalar.copy(gsb[:], gt[:])
                            g_sb.append(gsb)
                        o_ps = pso.tile([P, D], f32)
                        for k in range(2):
                            nc.tensor.matmul(
                                o_ps[:],
                                lhsT=g_sb[k][:],
                                rhs=wt_t[k][:],
                                start=(k == 0),
                                stop=(k == 1),
                            )
                        o_sb = osbp.tile([P, D], f32)
                        nc.scalar.copy(o_sb[:], o_ps[:])
                        rows = min(P, sh - dbi * P)
                        nc.sync.dma_start(
                            out=out_d[dbi * P : dbi * P + rows, :],
                            in_=o_sb[:rows, :],
                        )

    nc.compile()
    return nc


def _make_in_maps(x, W, pl):
    xb = x.astype(BF16)
    wt = np.ascontiguousarray(W.T).astype(BF16)
    iota = np.tile(np.arange(P, dtype=np.float32), (P, 1)).astype(BF16)
    return [
        {
            "x": xb,
            "wt": wt,
            "iota": iota,
            "idx": pl.idx_hosts[m],
            "val": pl.val_hosts[m],
            "row": pl.row_hosts[m],
            "nval": pl.nval_hosts[m],
            "sh": pl.s_hosts[m],
        }
        for m in range(NCORES)
    ]


def _run(nc, in_maps):
    from concourse.bass_utils import run_bass_kernel_spmd

    res = run_bass_kernel_spmd(nc, in_maps, list(range(NCORES)))
    return np.concatenate([res.results[m]["out"] for m in range(NCORES)], axis=0)


def kernel(x, W, edge_vals, edge_row, edge_col):
    x = np.asarray(x, np.float32)
    W = np.asarray(W, np.float32)
    edge_vals = np.asarray(edge_vals, np.float32)
    edge_row = np.asarray(edge_row, np.int32)
    edge_col = np.asarray(edge_col, np.int32)

    pl = _prep(edge_row, edge_col, edge_vals)
    nc = _build(pl, reps=1)
    in_maps = _make_in_maps(x, W, pl)
    return _run(nc, in_maps)
